# revision 1
# baseline (speedup 1.0000x reference)
"""MoLA (top-2 MoE over rank-16 LoRA experts) Trainium2 kernel.

Strategy: token-data-parallel over 8 NeuronCores (1024 tokens each), weights
replicated.  All device-side matmuls run "feature-major": the host feeds each
core x^T [D=2048, 1024] so the contraction dim D lands on SBUF partitions with
no on-chip transposes of the big tensor.

Per core, per 256-token tile:
  logits^T [8,256]   = sum_c gwT_c.T @ xT_c          (16 accum matmuls)
  h        [128,256] = sum_c A_c.T   @ xT_c          (ER=8*16=128 partitions)
  token-major logits = logits^T-chunk.T @ I8         (PE transpose, M=128)
  routing: vector.max (top-8 sorted) + match_replace -> exact top-2 masks,
           w2 = sigmoid(l2-l1), combine = m1*(w1-w2) + m12*w2
  combine^T [8,256]  = combine-chunk.T @ I128        (PE transpose)
  cexp [128,256]     = S.T @ combine^T               (expand E -> E*R rows)
  hw = h * cexp                                       (DVE)
  out [128,512]x8    = hw-chunk.T @ B_all             (K=ER single block)
SCALING=2.0 is folded into B_all on the host (exact, power of two).
"""

import os
import sys

for _p in ("/opt/trn_rl_repo", "/root/.axon_site/_ro/trn_rl_repo"):
    if os.path.isdir(_p) and _p not in sys.path:
        sys.path.insert(0, _p)

import numpy as np

import concourse.bass as bass
import concourse.bacc as bacc
import concourse.mybir as mybir
from concourse.bass_utils import run_bass_kernel_spmd
from concourse.tile import TileContext

N_CORES = 8
B, S, D = 4, 2048, 2048
T_FULL = B * S                # 8192 tokens
TS = T_FULL // N_CORES        # 1024 tokens per core
E, R, O = 8, 16, 2048
ER = E * R                    # 128
TILE = 256                    # tokens per pipeline tile
NTILES = TS // TILE           # 4
NCH = D // 128                # 16 contraction chunks
NQ = TILE // 128              # 2 token chunks of 128 per tile
NOC = O // 512                # 4 output column chunks
NEG = -1.0e30
F32 = mybir.dt.float32

TRACE = False                 # set True (e.g. from test.py) to capture a profile
LAST_RESULTS = None           # stashed BassKernelResults for inspection

_cached_nc = None


def _build():
    nc = bacc.Bacc("TRN2", target_bir_lowering=False, debug=False,
                   num_devices=N_CORES)

    xt = nc.declare_dram_parameter("xt", [D, TS], F32, isOutput=False)
    gwt = nc.declare_dram_parameter("gwt", [D, E], F32, isOutput=False)
    a_all = nc.declare_dram_parameter("a_all", [D, ER], F32, isOutput=False)
    b_all = nc.declare_dram_parameter("b_all", [ER, O], F32, isOutput=False)
    smat = nc.declare_dram_parameter("smat", [E, ER], F32, isOutput=False)
    i8 = nc.declare_dram_parameter("i8", [E, E], F32, isOutput=False)
    i128 = nc.declare_dram_parameter("i128", [128, 128], F32, isOutput=False)
    out = nc.declare_dram_parameter("out", [TS, O], F32, isOutput=True)

    xt_r = xt.ap().rearrange("(c p) t -> p c t", p=128)       # [128, 16, TS]
    gwt_r = gwt.ap().rearrange("(c p) e -> p c e", p=128)     # [128, 16, 8]
    a_r = a_all.ap().rearrange("(c p) m -> p c m", p=128)     # [128, 16, 128]
    out_r = out.ap().rearrange("(t q p) o -> t p q o", p=128, q=NQ)

    with TileContext(nc) as tc:
        with (
            tc.tile_pool(name="const", bufs=1) as cpool,
            tc.tile_pool(name="work", bufs=2) as wpool,
            tc.tile_pool(name="rt", bufs=4) as rpool,
            tc.tile_pool(name="outp", bufs=3) as opool,
            tc.tile_pool(name="ps_lg", bufs=1, space="PSUM") as pslg,
            tc.tile_pool(name="ps_h", bufs=2, space="PSUM") as psh,
            tc.tile_pool(name="ps_ltm", bufs=1, space="PSUM") as psltm,
            tc.tile_pool(name="ps_ct", bufs=1, space="PSUM") as psct,
            tc.tile_pool(name="ps_cx", bufs=1, space="PSUM") as pscx,
            tc.tile_pool(name="ps_out", bufs=2, space="PSUM") as psout,
        ):
            # ---- resident weights / constants -------------------------------
            gwt_sb = cpool.tile([128, NCH, E], F32)
            nc.sync.dma_start(out=gwt_sb, in_=gwt_r)
            a_sb = cpool.tile([128, NCH, ER], F32)
            nc.sync.dma_start(out=a_sb, in_=a_r)
            b_sb = cpool.tile([ER, O], F32)
            nc.sync.dma_start(out=b_sb, in_=b_all.ap())
            s_sb = cpool.tile([E, ER], F32)
            nc.sync.dma_start(out=s_sb, in_=smat.ap())
            i8_sb = cpool.tile([E, E], F32)
            nc.sync.dma_start(out=i8_sb, in_=i8.ap())
            i128_sb = cpool.tile([128, 128], F32)
            nc.sync.dma_start(out=i128_sb, in_=i128.ap())

            # ---- stream x^T in per-tile slabs (2 MiB per dma) ---------------
            xt_sb = cpool.tile([128, NCH, TS], F32)
            for t in range(NTILES):
                nc.sync.dma_start(
                    out=xt_sb[:, :, t * TILE:(t + 1) * TILE],
                    in_=xt_r[:, :, t * TILE:(t + 1) * TILE],
                )

            for t in range(NTILES):
                tsl = slice(t * TILE, (t + 1) * TILE)

                # gate logits^T [8, TILE] and h [128, TILE]
                ps_lg = pslg.tile([E, TILE], F32)
                ps_h = psh.tile([128, TILE], F32)
                for c in range(NCH):
                    nc.tensor.matmul(ps_lg, gwt_sb[:, c, :], xt_sb[:, c, tsl],
                                     start=(c == 0), stop=(c == NCH - 1))
                for c in range(NCH):
                    nc.tensor.matmul(ps_h, a_sb[:, c, :], xt_sb[:, c, tsl],
                                     start=(c == 0), stop=(c == NCH - 1))

                lgT = wpool.tile([E, TILE], F32, tag="lgT")
                nc.scalar.copy(lgT, ps_lg)

                # logits to token-major [128, NQ, 8] via PE transpose
                ps_ltm = psltm.tile([128, NQ, E], F32)
                for q in range(NQ):
                    nc.tensor.matmul(ps_ltm[:, q, :],
                                     lgT[:, q * 128:(q + 1) * 128], i8_sb)
                ltm = wpool.tile([128, NQ, E], F32, tag="ltm")
                nc.scalar.copy(ltm, ps_ltm)

                # routing math -> combine [128, NQ, 8]
                comb = wpool.tile([128, NQ, E], F32, tag="comb")
                for q in range(NQ):
                    lq = ltm[:, q, :]
                    maxs = rpool.tile([128, 8], F32, tag="maxs")
                    nc.vector.max(out=maxs, in_=lq)
                    d = rpool.tile([128, 1], F32, tag="d")
                    nc.vector.tensor_sub(d, maxs[:, 1:2], maxs[:, 0:1])
                    w2 = rpool.tile([128, 1], F32, tag="w2")
                    nc.scalar.activation(w2, d,
                                         mybir.ActivationFunctionType.Sigmoid)
                    w1m2 = rpool.tile([128, 1], F32, tag="w1m2")
                    # w1 - w2 = 1 - 2*w2
                    nc.scalar.activation(w1m2, w2,
                                         mybir.ActivationFunctionType.Copy,
                                         bias=1.0, scale=-2.0)
                    scr2 = rpool.tile([128, 8], F32, tag="scr2")
                    nc.vector.memset(scr2, NEG)
                    nc.vector.tensor_copy(scr2[:, 0:2], maxs[:, 0:2])
                    lm2 = rpool.tile([128, 8], F32, tag="lm2")
                    nc.vector.match_replace(out=lm2, in_to_replace=scr2,
                                            in_values=lq, imm_value=NEG)
                    scr1 = rpool.tile([128, 8], F32, tag="scr1")
                    nc.vector.memset(scr1, NEG)
                    nc.vector.tensor_copy(scr1[:, 0:1], maxs[:, 0:1])
                    lm1 = rpool.tile([128, 8], F32, tag="lm1")
                    nc.vector.match_replace(out=lm1, in_to_replace=scr1,
                                            in_values=lq, imm_value=NEG)
                    mask1 = rpool.tile([128, 8], F32, tag="mask1")
                    nc.vector.tensor_scalar(mask1, lm1, NEG, None,
                                            op0=mybir.AluOpType.is_equal)
                    mask12 = rpool.tile([128, 8], F32, tag="mask12")
                    nc.vector.tensor_scalar(mask12, lm2, NEG, None,
                                            op0=mybir.AluOpType.is_equal)
                    t1 = rpool.tile([128, 8], F32, tag="t1")
                    nc.vector.tensor_scalar(t1, mask1, w1m2, None,
                                            op0=mybir.AluOpType.mult)
                    t2 = rpool.tile([128, 8], F32, tag="t2")
                    nc.vector.tensor_scalar(t2, mask12, w2, None,
                                            op0=mybir.AluOpType.mult)
                    nc.vector.tensor_add(comb[:, q, :], t1, t2)

                # combine^T [8, TILE] via PE transpose, then expand to [128, TILE]
                ps_ct = psct.tile([E, TILE], F32)
                for q in range(NQ):
                    nc.tensor.matmul(ps_ct[:, q * 128:(q + 1) * 128],
                                     comb[:, q, :], i128_sb)
                cT = wpool.tile([E, TILE], F32, tag="cT")
                nc.scalar.copy(cT, ps_ct)
                ps_cx = pscx.tile([128, TILE], F32)
                nc.tensor.matmul(ps_cx, s_sb, cT)
                cx = wpool.tile([128, TILE], F32, tag="cx")
                nc.scalar.copy(cx, ps_cx)

                hw = wpool.tile([128, TILE], F32, tag="hw")
                nc.vector.tensor_mul(hw, ps_h, cx)

                # out [TILE, O] = (hw*combine).T @ B_all, K = ER = 128
                osb = opool.tile([128, NQ, O], F32, tag="osb")
                for q in range(NQ):
                    for oc in range(NOC):
                        ps_o = psout.tile([128, 512], F32)
                        nc.tensor.matmul(ps_o, hw[:, q * 128:(q + 1) * 128],
                                         b_sb[:, oc * 512:(oc + 1) * 512])
                        nc.scalar.copy(osb[:, q, oc * 512:(oc + 1) * 512], ps_o)
                nc.sync.dma_start(out=out_r[t], in_=osb)

    nc.finalize()
    return nc


def _get_nc():
    global _cached_nc
    if _cached_nc is None:
        _cached_nc = _build()
    return _cached_nc


def _host_prep(x, gate_w, lora_A, lora_B):
    xf = np.ascontiguousarray(np.asarray(x, dtype=np.float32)).reshape(T_FULL, D)
    gwt = np.ascontiguousarray(np.asarray(gate_w, dtype=np.float32).T)
    a_all = np.ascontiguousarray(
        np.asarray(lora_A, dtype=np.float32).transpose(2, 0, 1).reshape(D, ER))
    b_all = np.ascontiguousarray(
        np.asarray(lora_B, dtype=np.float32).transpose(0, 2, 1).reshape(ER, O)
        * np.float32(2.0))
    smat = np.zeros((E, ER), dtype=np.float32)
    for e in range(E):
        smat[e, e * R:(e + 1) * R] = 1.0
    i8 = np.eye(E, dtype=np.float32)
    i128 = np.eye(128, dtype=np.float32)
    in_maps = []
    for i in range(N_CORES):
        xts = np.ascontiguousarray(xf[i * TS:(i + 1) * TS, :].T)
        in_maps.append({"xt": xts, "gwt": gwt, "a_all": a_all, "b_all": b_all,
                        "smat": smat, "i8": i8, "i128": i128})
    return in_maps


def kernel(x, gate_w, lora_A, lora_B):
    global LAST_RESULTS
    nc = _get_nc()
    in_maps = _host_prep(x, gate_w, lora_A, lora_B)
    res = run_bass_kernel_spmd(nc, in_maps, list(range(N_CORES)), trace=TRACE)
    LAST_RESULTS = res
    shards = [res.results[i]["out"] for i in range(N_CORES)]
    return np.concatenate(shards, axis=0).reshape(B, S, O)

